# revision 26
# baseline (speedup 1.0000x reference)
"""Causal MHA + out-projection Trainium2 kernel (B=64, S=512, E=256, H=4).

Sharding: data-parallel over batch, 8 batches per NeuronCore x 8 cores.

v2 — engine-balanced restructure of the v1 kernel:
  - QK^T emitted as interleaved head-PAIR matmuls: heads 2i/2i+1 live on
    kt/qt partition halves 0:64 / 64:128, so their K=64 matmuls occupy
    disjoint PE row groups (h0/h64), run concurrently, and let every
    LDWEIGHTS pull ahead of the in-flight matmul (no weight-load bubble).
  - causal masking moved OFF the PE: exp() runs on raw logits, then two
    gpsimd affine_selects zero the upper triangles of the four diagonal
    128x128 blocks of P^T (cols {0,512} and {896,1024}).
  - softmax normalize is 2 DVE ops/head: reciprocal_approx_fast straight
    from the PSUM denominator rows, then one multiply writing bf16 X^T.
  - out-proj bias + PSUM egress + bf16 cast fused into one DVE
    scalar_tensor_tensor per q-pair (bias pre-broadcast to 128 partitions
    by DMA); no bias matmuls, no ACT egress copy.
  - output stored bf16 (cast to fp32 on host) to halve the DMA tail.

Host-side prep: qry/key are pre-transposed to [B, E, S] and w_out to
W^T so that every device DMA is contiguous.
"""

import sys

if "/opt/trn_rl_repo" not in sys.path:
    sys.path.insert(0, "/opt/trn_rl_repo")

import numpy as np

import concourse.bass as bass
import concourse.mybir as mybir
import concourse.tile as tile
from concourse import bacc
from concourse.bass_utils import run_bass_kernel_spmd

B, S, E, H = 64, 512, 256, 4
D = E // H  # 64
P = 128
NCORES = 8
BPC = B // NCORES  # 8

FP = mybir.dt.float32
BF = mybir.dt.bfloat16
TD = BF

# S^T chunk -> column offset inside the 3-bank (1536 col) psum tile.
# chunk j covers keys [128j, 128j+128), q in [128j, 512) => width 512-128j.
# Diagonal blocks land at cols {0, 512} (chunks 0,1) and {896, 1024}
# (chunks 3,2) — two uniform-stride groups for the gpsimd mask selects.
CHUNK_COL = [0, 512, 1024, 896]
ST_COLS = 1280

import os

# bisect toggles: set to "v1" to use the baseline-style implementation
OPT_EGRESS = os.environ.get("OPT_EGRESS", "fused")  # fused | v1
OPT_RECIP = os.environ.get("OPT_RECIP", "swap")  # swap | copy | direct | alignhi
OPT_MASK = os.environ.get("OPT_MASK", "gpsimd")  # gpsimd | matmul
NEG = -1.0e10


def attention_kernel(ctx, tc, out, qT, kT, v, wT, bo):
    nc = tc.nc
    AF = mybir.ActivationFunctionType
    OP = mybir.AluOpType

    consts = ctx.enter_context(tc.tile_pool(name="consts", bufs=1))
    qk_pool = ctx.enter_context(tc.tile_pool(name="qk", bufs=2))
    v_pool = ctx.enter_context(tc.tile_pool(name="v", bufs=2))
    pt_pool = ctx.enter_context(tc.tile_pool(name="pt", bufs=3))
    xt_pool = ctx.enter_context(tc.tile_pool(name="xt", bufs=2))
    yo_pool = ctx.enter_context(tc.tile_pool(name="yo", bufs=2))
    r_pool = ctx.enter_context(tc.tile_pool(name="r", bufs=3))
    st_psum = ctx.enter_context(tc.tile_pool(name="st", bufs=2, space="PSUM"))
    # ot ([128,512]) and y ([128,2,256]) share one 1-bank slot tag so the
    # PV->normalize chain gets 2 buffers without exceeding 8 psum banks.
    oy_psum = ctx.enter_context(tc.tile_pool(name="oy", bufs=2, space="PSUM"))

    # ---- constants ----
    # W^T as out-proj rhs: [c-part, c-chunk, e].  The DMAs are emitted
    # after the first batch's loads (deferred) so they don't delay the
    # first QK matmuls.
    wt_sb = consts.tile([P, 2, E], TD)
    bias_rep = consts.tile([P, E], FP, tag="bias_rep")

    def emit_const_dmas():
        nc.sync.dma_start(wt_sb[:], wT.rearrange("(c p) e -> p c e", p=P))
        nc.sync.dma_start(bias_rep[:], bo.broadcast_to([P, E]))

    if OPT_EGRESS == "v1":
        bias_f = consts.tile([1, E], FP, tag="bias_f")
        nc.sync.dma_start(bias_f[:], bo)
        brow = consts.tile([1, E], BF)
        nc.vector.tensor_copy(brow[:], bias_f[:])
        ones_f = consts.tile([1, P], FP, tag="ones_f")
        nc.gpsimd.memset(ones_f[:], 1.0)
        ones1 = consts.tile([1, P], BF)
        nc.vector.tensor_copy(ones1[:], ones_f[:])

    if OPT_MASK == "matmul":
        mf = consts.tile([P, P], FP, tag="mf")
        a_bf = consts.tile([P, P], BF)
        b_bf = consts.tile([P, P], BF)
        nc.gpsimd.memset(mf[:], 1.0)
        nc.gpsimd.affine_select(
            out=mf[:], in_=mf[:], compare_op=mybir.AluOpType.is_ge, fill=0.0,
            base=-1, pattern=[[1, P]], channel_multiplier=-1,
        )
        nc.vector.tensor_copy(a_bf[:], mf[:])
        mf2 = consts.tile([P, P], FP, tag="mf2")
        nc.gpsimd.memset(mf2[:], NEG)
        nc.gpsimd.affine_select(
            out=mf2[:], in_=mf2[:], compare_op=mybir.AluOpType.is_ge, fill=0.0,
            base=0, pattern=[[-1, P]], channel_multiplier=1,
        )
        nc.vector.tensor_copy(b_bf[:], mf2[:])
        consts_mask = (a_bf, b_bf)
    else:
        consts_mask = None

    # ---- software-pipelined emission over (batch, head-pair) units ----
    batch_states = {}

    def emit_loads(b):
        # per-(head-parity-chunk) kt/qt tiles so the first pair's QK only
        # waits on the first half of the load.  kt1/qt1 are issued from the
        # scalar engine so the two descriptor writes proceed in parallel
        # with sync's (each issue costs ~640ns on the issuing engine).
        qt0 = qk_pool.tile([P, S], TD, tag="qt0", name=f"qt0_{b}")
        kt0 = qk_pool.tile([P, S], TD, tag="kt0", name=f"kt0_{b}")
        qt1 = qk_pool.tile([P, S], TD, tag="qt1", name=f"qt1_{b}")
        kt1 = qk_pool.tile([P, S], TD, tag="kt1", name=f"kt1_{b}")
        nc.sync.dma_start(kt0[:], kT[b, 0:P])
        nc.sync.dma_start(qt0[:], qT[b, 0:P])
        nc.scalar.dma_start(kt1[:], kT[b, P:E])
        nc.scalar.dma_start(qt1[:], qT[b, P:E])
        va = v_pool.tile([P, 4, H, P], TD, tag="va", name=f"va{b}")
        if OPT_RECIP == "swap":
            # ones in cols 0:64 so the PV denominator lands on psum
            # partitions 0:64 where the recip can read it base-aligned.
            # The DMA only writes cols 64:128, so the ones written into
            # each of the pool's buffers on first use survive reuse.
            if b < 2:
                nc.gpsimd.memset(va[:, :, :, :D], 1.0)
            voff = D
        else:
            if b < 2:
                nc.gpsimd.memset(va[:, :, :, D:], 1.0)
            voff = 0
        for j in range(4):
            nc.sync.dma_start(
                va[:, j, :, voff : voff + D],
                v[b, P * j : P * (j + 1)].rearrange("p (h d) -> p h d", d=D),
            )
        xt_sb = xt_pool.tile([P, 2, S], TD, tag="xt", name=f"xt{b}")
        batch_states[b] = ((qt0, kt0), (qt1, kt1), va, xt_sb)

    def emit_qk_pair(b, i):
        # heads hA=2i (rows 0:64) and hB=2i+1 (rows 64:128) of qt/kt chunk i,
        # interleaved so consecutive MMs alternate PE row groups.
        if i == 0:
            emit_loads(b)
        qtkt0, qtkt1, va, xt_sb = batch_states[b]
        qt, kt = (qtkt0, qtkt1)[i]
        stA = st_psum.tile([P, ST_COLS], FP, tag="st", name=f"stA{b}_{i}")
        stB = st_psum.tile([P, ST_COLS], FP, tag="st", name=f"stB{b}_{i}")
        for j in range(4):
            qoff = P * j
            qr = S - qoff
            col = CHUNK_COL[j]
            for st, hp in ((stA, 0), (stB, D)):
                nc.tensor.matmul(
                    st[:, col : col + qr],
                    lhsT=kt[hp : hp + D, qoff : qoff + P],
                    rhs=qt[hp : hp + D, qoff:S],
                    start=True,
                    stop=(OPT_MASK != "matmul"),
                )
            if OPT_MASK == "matmul":
                a_bf, b_bf = consts_mask
                for st in (stA, stB):
                    nc.tensor.matmul(
                        st[:, col : col + P],
                        lhsT=a_bf[:],
                        rhs=b_bf[:],
                        start=False,
                        stop=True,
                    )
        return (b, i, stA, stB)

    def emit_tail_head(b, i, hp, st):
        # exp -> mask -> PV -> normalize for one head (h = 2i + hp//D)
        _, _, va, xt_sb = batch_states[b]
        h = 2 * i + (1 if hp else 0)
        pt = pt_pool.tile([P, ST_COLS], TD, tag="pt", name=f"pt{b}_{h}")
        nc.scalar.activation(pt[:], st[:, 0:ST_COLS], AF.Exp, scale=0.125)

        # zero the masked upper triangle of each diagonal 128x128 block:
        # keep iff q_local - k >= 0.  Diag blocks sit at cols {0,512} and
        # {896,1024} — two uniform-stride [p, 2, 128] views.
        if OPT_MASK == "gpsimd":
            for base, step in ((0, 512), (896, 128)):
                blk = pt[:, base : base + 2 * step].rearrange(
                    "p (g c) -> p g c", c=step
                )[:, :, 0:P]
                nc.gpsimd.affine_select(
                    out=blk,
                    in_=blk,
                    compare_op=OP.is_ge,
                    fill=0.0,
                    base=0,
                    pattern=[[0, 2], [1, P]],
                    channel_multiplier=-1,
                )

        ot = oy_psum.tile([P, S], FP, tag="oy", name=f"ot{b}_{h}")
        for j in range(4):
            qoff = P * j
            qr = S - qoff
            col = CHUNK_COL[j]
            nc.tensor.matmul(
                ot[:, qoff:S],
                lhsT=va[:, j, h, :],
                rhs=pt[:, col : col + qr],
                start=(j == 0),
                stop=(j == 3),
            )

        # normalize: rows 0:64 O^T, rows 64:128 denom replicated 64x.
        hc = i
        if OPT_RECIP == "swap":
            # va has ones first: rows 0:64 = denom, rows 64:128 = O^T
            r_sb = r_pool.tile([D, S], FP, tag="rsb", name=f"r{b}_{h}")
            nc.vector.reciprocal_approx_fast(r_sb[:], ot[0:D, :])
            nc.vector.tensor_tensor(
                xt_sb[hp : hp + D, hc, :], ot[D:P, :], r_sb[:], OP.mult
            )
            return
        if OPT_RECIP == "alignhi":
            # recip in/out both on partitions 64:128 (custom DVE op needs
            # aligned operands); the regular mult reads r misaligned.
            r2 = r_pool.tile([P, S], FP, tag="rsb", name=f"r{b}_{h}")
            nc.vector.reciprocal_approx_fast(r2[D:P, :], ot[D:P, :])
            nc.vector.tensor_tensor(
                xt_sb[hp : hp + D, hc, :], ot[0:D, :], r2[D:P, :], OP.mult
            )
            return
        r_sb = r_pool.tile([D, S], FP, tag="rsb", name=f"r{b}_{h}")
        if OPT_RECIP == "copy":
            d_sb = r_pool.tile([D, S], FP, tag="dsb", name=f"d{b}_{h}")
            nc.vector.tensor_copy(d_sb[:], ot[D:P, :])
            nc.vector.reciprocal_approx_fast(r_sb[:], d_sb[:])
        elif OPT_RECIP == "gpcopy":
            d_sb = r_pool.tile([D, S], FP, tag="dsb", name=f"d{b}_{h}")
            nc.gpsimd.tensor_copy(d_sb[:], ot[D:P, :])
            nc.vector.reciprocal_approx_fast(r_sb[:], d_sb[:])
        else:
            nc.vector.reciprocal_approx_fast(r_sb[:], ot[D:P, :])
        nc.vector.tensor_tensor(
            xt_sb[hp : hp + D, hc, :], ot[0:D, :], r_sb[:], OP.mult
        )

    def emit_tail(state):
        b, i, stA, stB = state
        emit_tail_head(b, i, 0, stA)
        emit_tail_head(b, i, D, stB)
        return b if i == 1 else None

    def emit_proj(b):
        _, _, va, xt_sb = batch_states[b]
        yout = yo_pool.tile([P, 4, E], TD, tag="yout", name=f"yo{b}")
        for qp in range(2):
            y = oy_psum.tile([P, 2, E], FP, tag="oy", name=f"y{b}_{qp}")
            for qi in range(2):
                qt_idx = 2 * qp + qi
                for c in range(2):
                    nc.tensor.matmul(
                        y[:, qi, :],
                        lhsT=xt_sb[:, c, P * qt_idx : P * (qt_idx + 1)],
                        rhs=wt_sb[:, c, :],
                        start=(c == 0),
                        stop=(c == 1) and OPT_EGRESS != "v1",
                    )
                if OPT_EGRESS == "v1":
                    nc.tensor.matmul(
                        y[:, qi, :],
                        lhsT=ones1[:],
                        rhs=brow[:],
                        start=False,
                        stop=True,
                    )
            if OPT_EGRESS == "v1":
                nc.scalar.copy(yout[:, 2 * qp : 2 * qp + 2, :], y[:])
            else:
                # fused: psum read + bias add + bf16 cast
                nc.vector.scalar_tensor_tensor(
                    out=yout[:, 2 * qp : 2 * qp + 2, :],
                    in0=y[:],
                    scalar=1.0,
                    in1=bias_rep.unsqueeze(1).broadcast_to([P, 2, E]),
                    op0=OP.bypass,
                    op1=OP.add,
                )
        # issue the store from gpsimd so it neither waits on nor occupies
        # the sync DMA queue (only SP/ACT/gpsimd can initiate DMAs)
        nc.gpsimd.dma_start(out[b].rearrange("(g p) e -> p g e", p=P), yout[:])
        del batch_states[b]

    units = [(b, i) for b in range(BPC) for i in range(2)]
    pending = None
    pending_proj = None
    for iu, bu in enumerate(units):
        state = emit_qk_pair(*bu)
        if iu == 0:
            emit_const_dmas()
        if pending is not None:
            pb = emit_tail(pending)
            if pending_proj is not None:
                emit_proj(pending_proj)
                pending_proj = None
            if pb is not None:
                pending_proj = pb
        pending = state
    pb = emit_tail(pending)
    if pending_proj is not None:
        emit_proj(pending_proj)
    if pb is not None:
        emit_proj(pb)


def build_nc(bpc=BPC):
    from contextlib import ExitStack

    nc = bacc.Bacc("TRN2", target_bir_lowering=False, debug=False)
    qT = nc.dram_tensor("qT", [bpc, E, S], TD, kind="ExternalInput").ap()
    kT = nc.dram_tensor("kT", [bpc, E, S], TD, kind="ExternalInput").ap()
    v = nc.dram_tensor("v", [bpc, S, E], TD, kind="ExternalInput").ap()
    wT = nc.dram_tensor("wT", [E, E], TD, kind="ExternalInput").ap()
    bo = nc.dram_tensor("bo", [1, E], FP, kind="ExternalInput").ap()
    out = nc.dram_tensor("out", [bpc, S, E], TD, kind="ExternalOutput").ap()

    with tile.TileContext(nc) as tc:
        with ExitStack() as ctx:
            saved = globals()["BPC"]
            globals()["BPC"] = bpc
            try:
                attention_kernel(ctx, tc, out, qT, kT, v, wT, bo)
            finally:
                globals()["BPC"] = saved
    nc.compile()
    return nc


def _np_td():
    import ml_dtypes

    return np.dtype(ml_dtypes.bfloat16)


def make_in_maps(qry, key, val, w_out, b_out):
    td = _np_td()
    qT_all = np.ascontiguousarray(qry.transpose(0, 2, 1)).astype(td)
    kT_all = np.ascontiguousarray(key.transpose(0, 2, 1)).astype(td)
    val = val.astype(td)
    wT = np.ascontiguousarray(w_out.T).astype(td)
    bo = np.ascontiguousarray(b_out.reshape(1, E))
    maps = []
    for c in range(NCORES):
        sl = slice(c * BPC, (c + 1) * BPC)
        maps.append(
            {
                "qT": qT_all[sl],
                "kT": kT_all[sl],
                "v": np.ascontiguousarray(val[sl]),
                "wT": wT,
                "bo": bo,
            }
        )
    return maps


_NC_CACHE = {}


def _get_nc():
    if "nc" not in _NC_CACHE:
        _NC_CACHE["nc"] = build_nc()
    return _NC_CACHE["nc"]


def kernel(qry, key, val, w_out, b_out, **run_kwargs):
    nc = _get_nc()
    in_maps = make_in_maps(
        np.asarray(qry, dtype=np.float32),
        np.asarray(key, dtype=np.float32),
        np.asarray(val, dtype=np.float32),
        np.asarray(w_out, dtype=np.float32),
        np.asarray(b_out, dtype=np.float32),
    )
    res = run_bass_kernel_spmd(nc, in_maps, core_ids=list(range(NCORES)), **run_kwargs)
    out = np.concatenate(
        [res.results[c]["out"].astype(np.float32) for c in range(NCORES)], axis=0
    )
    if run_kwargs:
        kernel.last_results = res
    return out


# revision 28
# speedup vs baseline: 1.0151x; 1.0151x over previous
"""Causal MHA + out-projection Trainium2 kernel (B=64, S=512, E=256, H=4).

Sharding: data-parallel over batch, 8 batches per NeuronCore x 8 cores.

v2 — engine-balanced restructure of the v1 kernel:
  - QK^T emitted as interleaved head-PAIR matmuls: heads 2i/2i+1 live on
    kt/qt partition halves 0:64 / 64:128, so their K=64 matmuls occupy
    disjoint PE row groups (h0/h64), run concurrently, and let every
    LDWEIGHTS pull ahead of the in-flight matmul (no weight-load bubble).
  - causal masking moved OFF the PE: exp() runs on raw logits, then two
    gpsimd affine_selects zero the upper triangles of the four diagonal
    128x128 blocks of P^T (cols {0,512} and {896,1024}).
  - softmax normalize is 2 DVE ops/head: reciprocal_approx_fast straight
    from the PSUM denominator rows, then one multiply writing bf16 X^T.
  - out-proj bias + PSUM egress + bf16 cast fused into one DVE
    scalar_tensor_tensor per q-pair (bias pre-broadcast to 128 partitions
    by DMA); no bias matmuls, no ACT egress copy.
  - output stored bf16 (cast to fp32 on host) to halve the DMA tail.

Host-side prep: qry/key are pre-transposed to [B, E, S] and w_out to
W^T so that every device DMA is contiguous.
"""

import sys

if "/opt/trn_rl_repo" not in sys.path:
    sys.path.insert(0, "/opt/trn_rl_repo")

import numpy as np

import concourse.bass as bass
import concourse.mybir as mybir
import concourse.tile as tile
from concourse import bacc
from concourse.bass_utils import run_bass_kernel_spmd

B, S, E, H = 64, 512, 256, 4
D = E // H  # 64
P = 128
NCORES = 8
BPC = B // NCORES  # 8

FP = mybir.dt.float32
BF = mybir.dt.bfloat16
TD = BF

# S^T chunk -> column offset inside the 3-bank (1536 col) psum tile.
# chunk j covers keys [128j, 128j+128), q in [128j, 512) => width 512-128j.
# Diagonal blocks land at cols {0, 512} (chunks 0,1) and {896, 1024}
# (chunks 3,2) — two uniform-stride groups for the gpsimd mask selects.
CHUNK_COL = [0, 512, 1024, 896]
ST_COLS = 1280

import os

# bisect toggles: set to "v1" to use the baseline-style implementation
OPT_EGRESS = os.environ.get("OPT_EGRESS", "fused")  # fused | v1
OPT_RECIP = os.environ.get("OPT_RECIP", "swap")  # swap | copy | direct | alignhi
OPT_MASK = os.environ.get("OPT_MASK", "gpsimd")  # gpsimd | matmul
NEG = -1.0e10


def attention_kernel(ctx, tc, out, qT, kT, v, wT, bo):
    nc = tc.nc
    AF = mybir.ActivationFunctionType
    OP = mybir.AluOpType

    consts = ctx.enter_context(tc.tile_pool(name="consts", bufs=1))
    qk_pool = ctx.enter_context(tc.tile_pool(name="qk", bufs=2))
    v_pool = ctx.enter_context(tc.tile_pool(name="v", bufs=2))
    pt_pool = ctx.enter_context(tc.tile_pool(name="pt", bufs=3))
    xt_pool = ctx.enter_context(tc.tile_pool(name="xt", bufs=2))
    yo_pool = ctx.enter_context(tc.tile_pool(name="yo", bufs=2))
    r_pool = ctx.enter_context(tc.tile_pool(name="r", bufs=3))
    st_psum = ctx.enter_context(tc.tile_pool(name="st", bufs=2, space="PSUM"))
    # ot ([128,512]) and y ([128,2,256]) share one 1-bank slot tag so the
    # PV->normalize chain gets 2 buffers without exceeding 8 psum banks.
    oy_psum = ctx.enter_context(tc.tile_pool(name="oy", bufs=2, space="PSUM"))

    # ---- constants ----
    # W^T as out-proj rhs: [c-part, c-chunk, e].  The DMAs are emitted
    # after the first batch's loads (deferred) so they don't delay the
    # first QK matmuls.
    wt_sb = consts.tile([P, 2, E], TD)
    bias_rep = consts.tile([P, E], FP, tag="bias_rep")

    def emit_const_dmas():
        nc.sync.dma_start(wt_sb[:], wT.rearrange("(c p) e -> p c e", p=P))
        nc.sync.dma_start(bias_rep[:], bo.broadcast_to([P, E]))

    if OPT_EGRESS == "v1":
        bias_f = consts.tile([1, E], FP, tag="bias_f")
        nc.sync.dma_start(bias_f[:], bo)
        brow = consts.tile([1, E], BF)
        nc.vector.tensor_copy(brow[:], bias_f[:])
        ones_f = consts.tile([1, P], FP, tag="ones_f")
        nc.gpsimd.memset(ones_f[:], 1.0)
        ones1 = consts.tile([1, P], BF)
        nc.vector.tensor_copy(ones1[:], ones_f[:])

    if OPT_MASK == "matmul":
        mf = consts.tile([P, P], FP, tag="mf")
        a_bf = consts.tile([P, P], BF)
        b_bf = consts.tile([P, P], BF)
        nc.gpsimd.memset(mf[:], 1.0)
        nc.gpsimd.affine_select(
            out=mf[:], in_=mf[:], compare_op=mybir.AluOpType.is_ge, fill=0.0,
            base=-1, pattern=[[1, P]], channel_multiplier=-1,
        )
        nc.vector.tensor_copy(a_bf[:], mf[:])
        mf2 = consts.tile([P, P], FP, tag="mf2")
        nc.gpsimd.memset(mf2[:], NEG)
        nc.gpsimd.affine_select(
            out=mf2[:], in_=mf2[:], compare_op=mybir.AluOpType.is_ge, fill=0.0,
            base=0, pattern=[[-1, P]], channel_multiplier=1,
        )
        nc.vector.tensor_copy(b_bf[:], mf2[:])
        consts_mask = (a_bf, b_bf)
    else:
        consts_mask = None

    # ---- software-pipelined emission over (batch, head-pair) units ----
    batch_states = {}

    def emit_loads(b):
        # per-(head-parity-chunk) kt/qt tiles so the first pair's QK only
        # waits on the first half of the load.  kt1/qt1 are issued from the
        # scalar engine so the two descriptor writes proceed in parallel
        # with sync's (each issue costs ~640ns on the issuing engine).
        qt0 = qk_pool.tile([P, S], TD, tag="qt0", name=f"qt0_{b}")
        kt0 = qk_pool.tile([P, S], TD, tag="kt0", name=f"kt0_{b}")
        qt1 = qk_pool.tile([P, S], TD, tag="qt1", name=f"qt1_{b}")
        kt1 = qk_pool.tile([P, S], TD, tag="kt1", name=f"kt1_{b}")
        nc.sync.dma_start(kt0[:], kT[b, 0:P])
        nc.sync.dma_start(qt0[:], qT[b, 0:P])
        nc.sync.dma_start(kt1[:], kT[b, P:E])
        nc.sync.dma_start(qt1[:], qT[b, P:E])
        va = v_pool.tile([P, 4, H, P], TD, tag="va", name=f"va{b}")
        if OPT_RECIP == "swap":
            # ones in cols 0:64 so the PV denominator lands on psum
            # partitions 0:64 where the recip can read it base-aligned.
            # The DMA only writes cols 64:128, so the ones written into
            # each of the pool's buffers on first use survive reuse.
            if b < 2:
                nc.gpsimd.memset(va[:, :, :, :D], 1.0)
            voff = D
        else:
            if b < 2:
                nc.gpsimd.memset(va[:, :, :, D:], 1.0)
            voff = 0
        for j in range(4):
            nc.sync.dma_start(
                va[:, j, :, voff : voff + D],
                v[b, P * j : P * (j + 1)].rearrange("p (h d) -> p h d", d=D),
            )
        xt_sb = xt_pool.tile([P, 2, S], TD, tag="xt", name=f"xt{b}")
        batch_states[b] = ((qt0, kt0), (qt1, kt1), va, xt_sb)

    def emit_qk_pair(b, i):
        # heads hA=2i (rows 0:64) and hB=2i+1 (rows 64:128) of qt/kt chunk i,
        # interleaved so consecutive MMs alternate PE row groups.
        if i == 0:
            emit_loads(b)
        qtkt0, qtkt1, va, xt_sb = batch_states[b]
        qt, kt = (qtkt0, qtkt1)[i]
        stA = st_psum.tile([P, ST_COLS], FP, tag="st", name=f"stA{b}_{i}")
        stB = st_psum.tile([P, ST_COLS], FP, tag="st", name=f"stB{b}_{i}")
        for j in range(4):
            qoff = P * j
            qr = S - qoff
            col = CHUNK_COL[j]
            for st, hp in ((stA, 0), (stB, D)):
                nc.tensor.matmul(
                    st[:, col : col + qr],
                    lhsT=kt[hp : hp + D, qoff : qoff + P],
                    rhs=qt[hp : hp + D, qoff:S],
                    start=True,
                    stop=(OPT_MASK != "matmul"),
                )
            if OPT_MASK == "matmul":
                a_bf, b_bf = consts_mask
                for st in (stA, stB):
                    nc.tensor.matmul(
                        st[:, col : col + P],
                        lhsT=a_bf[:],
                        rhs=b_bf[:],
                        start=False,
                        stop=True,
                    )
        return (b, i, stA, stB)

    def emit_tail_head(b, i, hp, st):
        # exp -> mask -> PV -> normalize for one head (h = 2i + hp//D)
        _, _, va, xt_sb = batch_states[b]
        h = 2 * i + (1 if hp else 0)
        pt = pt_pool.tile([P, ST_COLS], TD, tag="pt", name=f"pt{b}_{h}")
        nc.scalar.activation(pt[:], st[:, 0:ST_COLS], AF.Exp, scale=0.125)

        # zero the masked upper triangle of each diagonal 128x128 block:
        # keep iff q_local - k >= 0.  Diag blocks sit at cols {0,512} and
        # {896,1024} — two uniform-stride [p, 2, 128] views.
        if OPT_MASK == "gpsimd":
            for base, step in ((0, 512), (896, 128)):
                blk = pt[:, base : base + 2 * step].rearrange(
                    "p (g c) -> p g c", c=step
                )[:, :, 0:P]
                nc.gpsimd.affine_select(
                    out=blk,
                    in_=blk,
                    compare_op=OP.is_ge,
                    fill=0.0,
                    base=0,
                    pattern=[[0, 2], [1, P]],
                    channel_multiplier=-1,
                )

        ot = oy_psum.tile([P, S], FP, tag="oy", name=f"ot{b}_{h}")
        for j in range(4):
            qoff = P * j
            qr = S - qoff
            col = CHUNK_COL[j]
            nc.tensor.matmul(
                ot[:, qoff:S],
                lhsT=va[:, j, h, :],
                rhs=pt[:, col : col + qr],
                start=(j == 0),
                stop=(j == 3),
            )

        # normalize: rows 0:64 O^T, rows 64:128 denom replicated 64x.
        hc = i
        if OPT_RECIP == "swap":
            # va has ones first: rows 0:64 = denom, rows 64:128 = O^T
            r_sb = r_pool.tile([D, S], FP, tag="rsb", name=f"r{b}_{h}")
            nc.vector.reciprocal_approx_fast(r_sb[:], ot[0:D, :])
            nc.vector.tensor_tensor(
                xt_sb[hp : hp + D, hc, :], ot[D:P, :], r_sb[:], OP.mult
            )
            return
        if OPT_RECIP == "alignhi":
            # recip in/out both on partitions 64:128 (custom DVE op needs
            # aligned operands); the regular mult reads r misaligned.
            r2 = r_pool.tile([P, S], FP, tag="rsb", name=f"r{b}_{h}")
            nc.vector.reciprocal_approx_fast(r2[D:P, :], ot[D:P, :])
            nc.vector.tensor_tensor(
                xt_sb[hp : hp + D, hc, :], ot[0:D, :], r2[D:P, :], OP.mult
            )
            return
        r_sb = r_pool.tile([D, S], FP, tag="rsb", name=f"r{b}_{h}")
        if OPT_RECIP == "copy":
            d_sb = r_pool.tile([D, S], FP, tag="dsb", name=f"d{b}_{h}")
            nc.vector.tensor_copy(d_sb[:], ot[D:P, :])
            nc.vector.reciprocal_approx_fast(r_sb[:], d_sb[:])
        elif OPT_RECIP == "gpcopy":
            d_sb = r_pool.tile([D, S], FP, tag="dsb", name=f"d{b}_{h}")
            nc.gpsimd.tensor_copy(d_sb[:], ot[D:P, :])
            nc.vector.reciprocal_approx_fast(r_sb[:], d_sb[:])
        else:
            nc.vector.reciprocal_approx_fast(r_sb[:], ot[D:P, :])
        nc.vector.tensor_tensor(
            xt_sb[hp : hp + D, hc, :], ot[0:D, :], r_sb[:], OP.mult
        )

    def emit_tail(state):
        b, i, stA, stB = state
        emit_tail_head(b, i, 0, stA)
        emit_tail_head(b, i, D, stB)
        return b if i == 1 else None

    def emit_proj(b):
        _, _, va, xt_sb = batch_states[b]
        yout = yo_pool.tile([P, 4, E], TD, tag="yout", name=f"yo{b}")
        for qp in range(2):
            y = oy_psum.tile([P, 2, E], FP, tag="oy", name=f"y{b}_{qp}")
            for qi in range(2):
                qt_idx = 2 * qp + qi
                for c in range(2):
                    nc.tensor.matmul(
                        y[:, qi, :],
                        lhsT=xt_sb[:, c, P * qt_idx : P * (qt_idx + 1)],
                        rhs=wt_sb[:, c, :],
                        start=(c == 0),
                        stop=(c == 1) and OPT_EGRESS != "v1",
                    )
                if OPT_EGRESS == "v1":
                    nc.tensor.matmul(
                        y[:, qi, :],
                        lhsT=ones1[:],
                        rhs=brow[:],
                        start=False,
                        stop=True,
                    )
            if OPT_EGRESS == "v1":
                nc.scalar.copy(yout[:, 2 * qp : 2 * qp + 2, :], y[:])
            else:
                # fused: psum read + bias add + bf16 cast
                nc.vector.scalar_tensor_tensor(
                    out=yout[:, 2 * qp : 2 * qp + 2, :],
                    in0=y[:],
                    scalar=1.0,
                    in1=bias_rep.unsqueeze(1).broadcast_to([P, 2, E]),
                    op0=OP.bypass,
                    op1=OP.add,
                )
        nc.sync.dma_start(out[b].rearrange("(g p) e -> p g e", p=P), yout[:])
        del batch_states[b]

    units = [(b, i) for b in range(BPC) for i in range(2)]
    pending = None
    pending_proj = None
    for iu, bu in enumerate(units):
        state = emit_qk_pair(*bu)
        if iu == 0:
            emit_const_dmas()
        if pending is not None:
            pb = emit_tail(pending)
            if pending_proj is not None:
                emit_proj(pending_proj)
                pending_proj = None
            if pb is not None:
                pending_proj = pb
        pending = state
    pb = emit_tail(pending)
    if pending_proj is not None:
        emit_proj(pending_proj)
    if pb is not None:
        emit_proj(pb)


def build_nc(bpc=BPC):
    from contextlib import ExitStack

    nc = bacc.Bacc("TRN2", target_bir_lowering=False, debug=False)
    qT = nc.dram_tensor("qT", [bpc, E, S], TD, kind="ExternalInput").ap()
    kT = nc.dram_tensor("kT", [bpc, E, S], TD, kind="ExternalInput").ap()
    v = nc.dram_tensor("v", [bpc, S, E], TD, kind="ExternalInput").ap()
    wT = nc.dram_tensor("wT", [E, E], TD, kind="ExternalInput").ap()
    bo = nc.dram_tensor("bo", [1, E], FP, kind="ExternalInput").ap()
    out = nc.dram_tensor("out", [bpc, S, E], TD, kind="ExternalOutput").ap()

    with tile.TileContext(nc) as tc:
        with ExitStack() as ctx:
            saved = globals()["BPC"]
            globals()["BPC"] = bpc
            try:
                attention_kernel(ctx, tc, out, qT, kT, v, wT, bo)
            finally:
                globals()["BPC"] = saved
    nc.compile()
    return nc


def _np_td():
    import ml_dtypes

    return np.dtype(ml_dtypes.bfloat16)


def make_in_maps(qry, key, val, w_out, b_out):
    td = _np_td()
    qT_all = np.ascontiguousarray(qry.transpose(0, 2, 1)).astype(td)
    kT_all = np.ascontiguousarray(key.transpose(0, 2, 1)).astype(td)
    val = val.astype(td)
    wT = np.ascontiguousarray(w_out.T).astype(td)
    bo = np.ascontiguousarray(b_out.reshape(1, E))
    maps = []
    for c in range(NCORES):
        sl = slice(c * BPC, (c + 1) * BPC)
        maps.append(
            {
                "qT": qT_all[sl],
                "kT": kT_all[sl],
                "v": np.ascontiguousarray(val[sl]),
                "wT": wT,
                "bo": bo,
            }
        )
    return maps


_NC_CACHE = {}


def _get_nc():
    if "nc" not in _NC_CACHE:
        _NC_CACHE["nc"] = build_nc()
    return _NC_CACHE["nc"]


def kernel(qry, key, val, w_out, b_out, **run_kwargs):
    nc = _get_nc()
    in_maps = make_in_maps(
        np.asarray(qry, dtype=np.float32),
        np.asarray(key, dtype=np.float32),
        np.asarray(val, dtype=np.float32),
        np.asarray(w_out, dtype=np.float32),
        np.asarray(b_out, dtype=np.float32),
    )
    res = run_bass_kernel_spmd(nc, in_maps, core_ids=list(range(NCORES)), **run_kwargs)
    out = np.concatenate(
        [res.results[c]["out"].astype(np.float32) for c in range(NCORES)], axis=0
    )
    if run_kwargs:
        kernel.last_results = res
    return out
